# revision 14
# baseline (speedup 1.0000x reference)
"""Trainium2 Bass kernel for nn_GATSurvival (2-layer GAT + mean-pool + MLP).

Strategy (8 NeuronCores, SPMD):
- Nodes assigned to cores by balanced in-degree (snake deal over degree-sorted
  nodes) so per-core edge counts match; dst windows of 128 local nodes.
- Projection sharded: each core computes h1 = x_slab @ [W1 | W1@As | W1@Ad],
  writes 768B/row bf16 table [h1(256) | u=exp(as)(8) | u2=exp(.2as)(8) | pad],
  AllGathers the full table. dst-side exp(ad)/exp(.2ad) go straight into an
  SBUF-resident table (no DRAM roundtrip, no dst gather).
- Per-edge logits use exp(leaky(as+ad)) == max(exp(as)exp(ad), exp(.2as)exp(.2ad)).
- Only the src rows are dma_gathered (int16 idx, lo/hi table halves, two SWDGE
  queues alternated to pipeline descriptor generation against drain).
- Per-edge dst values come from a one-hot select matmul: St (edge x dst one-hot,
  built by is_equal) is transposed on the tensor engine, then StT.T @ dtab_win
  yields v per edge. Scatter-softmax/aggregation per 128-dst window via
  psum += St.T @ [alpha*h | alpha]; divide by the alpha-sum.
- Layer 2 mirrors layer 1 with 256B rows; graph mean-pool via a batch one-hot
  matmul accumulated in PSUM, AllReduce, and a tiny on-device MLP.
"""

import numpy as np
import ml_dtypes

import concourse.bass as bass
import concourse.mybir as mybir
from concourse import bacc
from concourse.tile import TileContext
from concourse.bass import ts
from concourse.bass_utils import run_bass_kernel_spmd
from concourse.masks import make_identity

bf16 = ml_dtypes.bfloat16
FP32 = mybir.dt.float32
BF16 = mybir.dt.bfloat16
I16 = mybir.dt.int16
U8 = mybir.dt.uint8
FP8 = mybir.dt.float8e4
AF = mybir.ActivationFunctionType
OP = mybir.AluOpType

NCORES, NLOCP = 8, 6272
NW = 49
HALF = 4 * NLOCP      # 25088
NFULL = 8 * NLOCP     # 50176
N, F, H, D = 50000, 256, 8, 32
G = 32
TCAP = 20             # max gather cols per chunk-half (ring: 8*TCAP+1 descs/engine)

last_results = None  # BassKernelResults of the most recent run (for test harness)


def _balance_nodes(dst_all):
    """LPT-pack nodes into the 8*NW (core, window) buckets balancing in-degree.

    Returns (node_core, node_local)."""
    import heapq
    deg = np.bincount(dst_all, minlength=N)
    order = np.argsort(-deg, kind="stable")
    nbuckets = NCORES * NW
    heap = [(0, b) for b in range(nbuckets)]
    heapq.heapify(heap)
    fill = np.zeros(nbuckets, np.int64)
    node_core = np.empty(N, np.int64)
    node_local = np.empty(N, np.int64)
    for n in order:
        load, b = heapq.heappop(heap)
        node_core[n] = b // NW
        node_local[n] = (b % NW) * 128 + fill[b]
        fill[b] += 1
        if fill[b] < 128:
            heapq.heappush(heap, (load + int(deg[n]), b))
    return node_core, node_local


def _prep_edges(edge_index):
    src = np.concatenate([np.asarray(edge_index[0]), np.arange(N)]).astype(np.int64)
    dst = np.concatenate([np.asarray(edge_index[1]), np.arange(N)]).astype(np.int64)
    node_core, node_local = _balance_nodes(dst)
    core = node_core[dst]
    dst_local = node_local[dst]
    table_row = node_core[src] * NLOCP + node_local[src]
    window = dst_local >> 7
    dst_rel = dst_local & 127
    is_hi = (table_row >= HALF).astype(np.int64)
    order = np.lexsort((is_hi, window, core))
    tr_s, drel_s = table_row[order], dst_rel[order]
    key = (core[order] * NW + window[order]) * 2 + is_hi[order]
    counts = np.bincount(key, minlength=NCORES * NW * 2).reshape(NCORES, NW, 2)
    T_LO = -(-counts[:, :, 0].max(axis=0) // 128)
    T_HI = -(-counts[:, :, 1].max(axis=0) // 128)
    # descriptor ring holds ~256 descs/engine; one gather needs 8*cols+1
    assert T_LO.max() <= 24 and T_HI.max() <= 24, (T_LO.max(), T_HI.max())
    # per-window select results go in one PSUM bank: Tw*16*4B <= 2KB
    assert (T_LO + T_HI).max() <= 32, (T_LO + T_HI).max()
    # greedy chunking: pack windows while both halves stay under TCAP columns
    chunks, cur, lo_sum, hi_sum = [], [], 0, 0
    for w in range(NW):
        if cur and (lo_sum + T_LO[w] > TCAP or hi_sum + T_HI[w] > TCAP):
            chunks.append(tuple(cur))
            cur, lo_sum, hi_sum = [], 0, 0
        cur.append(w)
        lo_sum += T_LO[w]
        hi_sum += T_HI[w]
    if cur:
        chunks.append(tuple(cur))
    starts = np.zeros(NCORES * NW * 2 + 1, dtype=np.int64)
    np.cumsum(counts.reshape(-1), out=starts[1:])
    percore = []
    for c in range(NCORES):
        lo_vals, hi_vals, dr_vals = [], [], []
        for ws in chunks:
            for w in ws:  # lo blocks first
                k = (c * NW + w) * 2
                ev = tr_s[starts[k]:starts[k + 1]]
                pad = T_LO[w] * 128 - len(ev)
                lo_vals.append(np.concatenate([ev, np.zeros(pad, np.int64)]))
                dr_vals.append(np.concatenate([drel_s[starts[k]:starts[k + 1]],
                                               np.full(pad, 255, np.int64)]))
            for w in ws:  # then hi blocks
                k = (c * NW + w) * 2 + 1
                ev = tr_s[starts[k]:starts[k + 1]] - HALF
                pad = T_HI[w] * 128 - len(ev)
                hi_vals.append(np.concatenate([ev, np.zeros(pad, np.int64)]))
                dr_vals.append(np.concatenate([drel_s[starts[k]:starts[k + 1]],
                                               np.full(pad, 255, np.int64)]))
        percore.append((np.concatenate(lo_vals), np.concatenate(hi_vals),
                        np.concatenate(dr_vals)))
    return T_LO, T_HI, chunks, percore, node_core, node_local


def _wrap_idx(vals):
    """[n] ints -> [128, n//16] int16 (16-partition wrap, replicated 8x)."""
    n = len(vals)
    w = vals.reshape(n // 16, 16).T.astype(np.int16)
    return np.ascontiguousarray(np.tile(w, (8, 1)))


def _chunk_cols(T_LO, T_HI, ws):
    """Per-window (lo_range, hi_range) column spans within a chunk."""
    spans = {}
    base = 0
    for w in ws:
        spans[w] = [(base, base + T_LO[w])]
        base += T_LO[w]
    for w in ws:
        spans[w].append((base, base + T_HI[w]))
        base += T_HI[w]
    return spans, base


def _build_program(T_LO, T_HI, chunks, CT, LOCOLS, HICOLS, c2_32, c2_33, bc2f):
    nc = bacc.Bacc(num_devices=NCORES, num_swdge_queues=2)

    xT = nc.dram_tensor("xT", [256, NLOCP], BF16, kind="ExternalInput")
    idxlo = nc.dram_tensor("idxlo", [128, LOCOLS], I16, kind="ExternalInput")
    idxhi = nc.dram_tensor("idxhi", [128, HICOLS], I16, kind="ExternalInput")
    dstrel = nc.dram_tensor("dstrel", [128, CT], BF16, kind="ExternalInput")
    sttoh = nc.dram_tensor("sttoh", [128, CT * 128], BF16, kind="ExternalInput")
    Sb = nc.dram_tensor("Sb", [NLOCP, 32], FP32, kind="ExternalInput")
    W1e = nc.dram_tensor("W1e", [256, 272], BF16, kind="ExternalInput")
    W2e = nc.dram_tensor("W2e", [256, 34], BF16, kind="ExternalInput")
    b1r = nc.dram_tensor("b1r", [128, 256], FP32, kind="ExternalInput")
    b2r = nc.dram_tensor("b2r", [128, 32], FP32, kind="ExternalInput")
    c2r = nc.dram_tensor("c2r", [128, 32], FP32, kind="ExternalInput")
    bc1t = nc.dram_tensor("bc1t", [16, 1], FP32, kind="ExternalInput")
    Wc1t = nc.dram_tensor("Wc1t", [32, 16], FP32, kind="ExternalInput")
    Wc2t = nc.dram_tensor("Wc2t", [16, 1], FP32, kind="ExternalInput")
    riskT = nc.dram_tensor("riskT", [1, 32], FP32, kind="ExternalOutput")

    h1x_mine = nc.dram_tensor("h1x_mine", [NLOCP, 512], U8, kind="Internal")
    h1x_full = nc.dram_tensor("h1x_full", [NFULL, 512], U8, kind="Internal",
                              addr_space="Shared")
    h2x_mine = nc.dram_tensor("h2x_mine", [NLOCP, 128], BF16, kind="Internal")
    h2x_full = nc.dram_tensor("h2x_full", [NFULL, 128], BF16, kind="Internal",
                              addr_space="Shared")
    pool_mine = nc.dram_tensor("pool_mine", [32, 33], FP32, kind="Internal")
    pool_sum = nc.dram_tensor("pool_sum", [32, 33], FP32, kind="Internal",
                              addr_space="Shared")

    rg = [list(range(NCORES))]

    with TileContext(nc) as tc:
        with tc.tile_pool(name="const", bufs=1) as cp:
            w1sb = cp.tile([128, 2, 272], BF16)
            nc.sync.dma_start(out=w1sb[:, 0, :], in_=W1e[0:128, :])
            nc.sync.dma_start(out=w1sb[:, 1, :], in_=W1e[128:256, :])
            w2sb = cp.tile([128, 2, 34], BF16)
            nc.sync.dma_start(out=w2sb[:, 0, :], in_=W2e[0:128, :])
            nc.sync.dma_start(out=w2sb[:, 1, :], in_=W2e[128:256, :])
            b1sb = cp.tile([128, 256], FP32)
            nc.sync.dma_start(out=b1sb[:], in_=b1r[:])
            b2sb = cp.tile([128, 32], FP32)
            nc.sync.dma_start(out=b2sb[:], in_=b2r[:])
            c2sb = cp.tile([128, 32], FP32)
            nc.sync.dma_start(out=c2sb[:], in_=c2r[:])
            bc1sb = cp.tile([16, 1], FP32)
            nc.sync.dma_start(out=bc1sb[:], in_=bc1t[:])
            wc1sb = cp.tile([32, 16], FP32)
            nc.sync.dma_start(out=wc1sb[:], in_=Wc1t[:])
            wc2sb = cp.tile([16, 1], FP32)
            nc.sync.dma_start(out=wc2sb[:], in_=Wc2t[:])
            drel_sb = cp.tile([128, CT], BF16)
            nc.sync.dma_start(out=drel_sb[:], in_=dstrel[:])
            iota_i = cp.tile([128, 128], mybir.dt.int32)
            nc.gpsimd.iota(iota_i[:], pattern=[[1, 128]], base=0, channel_multiplier=0)
            iota_bf = cp.tile([128, 128], BF16)
            nc.vector.tensor_copy(out=iota_bf[:], in_=iota_i[:])
            idf = cp.tile([128, 128], FP32)
            make_identity(nc, idf[:])
            idb = cp.tile([128, 128], BF16)
            nc.vector.tensor_copy(out=idb[:], in_=idf[:])
            bexp = cp.tile([128, 4], FP32)
            nc.vector.memset(bexp[:, 0:1], -c2_32)
            nc.vector.memset(bexp[:, 1:2], -0.2 * c2_32)
            nc.vector.memset(bexp[:, 2:3], -c2_33)
            nc.vector.memset(bexp[:, 3:4], -0.2 * c2_33)
            # SBUF-resident per-dst attention tables (written in P / E1 epilogue)
            dtab1 = cp.tile([128, NW, 16], BF16)
            dtab2 = cp.tile([128, NW, 2], BF16)

            # ---------------- Phase P: projection ----------------
            with tc.tile_pool(name="proj", bufs=3) as pp, \
                 tc.tile_pool(name="projps", bufs=2, space="PSUM") as ppp, \
                 tc.tile_pool(name="xs", bufs=3) as xp:
                for nt in range(NW):
                    xs = xp.tile([128, 2, 128], BF16, tag="xs")
                    nc.sync.dma_start(out=xs[:, 0, :], in_=xT[0:128, ts(nt, 128)])
                    nc.sync.dma_start(out=xs[:, 1, :], in_=xT[128:256, ts(nt, 128)])
                    ps = ppp.tile([128, 272], FP32, tag="ps")
                    nc.tensor.matmul(ps[:], lhsT=xs[:, 0, :],
                                     rhs=w1sb[:, 0, :], start=True, stop=False)
                    nc.tensor.matmul(ps[:], lhsT=xs[:, 1, :],
                                     rhs=w1sb[:, 1, :], start=False, stop=True)
                    hrow = pp.tile([128, 512], U8, tag="hrow")
                    nc.vector.tensor_copy(out=hrow[:, 0:256].bitcast(FP8),
                                          in_=ps[:, 0:256])
                    nc.scalar.activation(hrow[:, 256:272].bitcast(BF16),
                                         ps[:, 256:264], AF.Exp)
                    nc.scalar.activation(hrow[:, 272:288].bitcast(BF16),
                                         ps[:, 256:264], AF.Exp, scale=0.2)
                    nc.scalar.activation(dtab1[:, nt, 0:8], ps[:, 264:272], AF.Exp)
                    nc.scalar.activation(dtab1[:, nt, 8:16], ps[:, 264:272], AF.Exp,
                                         scale=0.2)
                    nc.sync.dma_start(out=h1x_mine[ts(nt, 128), :], in_=hrow[:])

            nc.gpsimd.collective_compute(
                "AllGather", OP.bypass, replica_groups=rg,
                ins=[h1x_mine[:]], outs=[h1x_full[:]])

            # ---------------- Phase E1: layer-1 edges ----------------
            with tc.tile_pool(name="e1", bufs=3) as e1, \
                 tc.tile_pool(name="e1s", bufs=2) as e1s, \
                 tc.tile_pool(name="e1t", bufs=3) as e1t, \
                 tc.tile_pool(name="e1ep", bufs=2) as ep, \
                 tc.tile_pool(name="e1tp", bufs=2, space="PSUM") as e1tp, \
                 tc.tile_pool(name="e1ps", bufs=2, space="PSUM") as e1ps:
                olo = ohi = 0
                ct0 = 0
                for s, ws in enumerate(chunks):
                    spans, C = _chunk_cols(T_LO, T_HI, ws)
                    ntl = sum(T_LO[w] for w in ws)
                    nlo, nhi = ntl * 128, (C - ntl) * 128
                    Gt = e1.tile([128, C, 512], U8, tag="G")
                    if nlo:
                        ilo = e1.tile([128, nlo // 16], I16, tag="ilo")
                        nc.sync.dma_start(out=ilo[:], in_=idxlo[:, olo:olo + nlo // 16])
                        nc.gpsimd.dma_gather(
                            out_ap=Gt[:, 0:ntl, :], in_ap=h1x_full[0:HALF, :],
                            idxs_ap=ilo[:], num_idxs=nlo, num_idxs_reg=nlo,
                            elem_size=512, single_packet=False, queue_num=0)
                    if nhi:
                        ihi = e1.tile([128, nhi // 16], I16, tag="ihi")
                        nc.sync.dma_start(out=ihi[:], in_=idxhi[:, ohi:ohi + nhi // 16])
                        nc.gpsimd.dma_gather(
                            out_ap=Gt[:, ntl:C, :], in_ap=h1x_full[HALF:NFULL, :],
                            idxs_ap=ihi[:], num_idxs=nhi, num_idxs_reg=nhi,
                            elem_size=512, single_packet=False, queue_num=1)
                    olo += nlo // 16; ohi += nhi // 16

                    St = e1s.tile([128, C, 128], BF16, tag="St")
                    nc.vector.tensor_tensor(
                        out=St[:],
                        in0=iota_bf[:][:, None, :].to_broadcast([128, C, 128]),
                        in1=drel_sb[:, ct0:ct0 + C][:, :, None].to_broadcast([128, C, 128]),
                        op=OP.is_equal)
                    SttC = e1s.tile([128, C, 128], BF16, tag="SttC")
                    nc.sync.dma_start(out=SttC[:],
                                      in_=sttoh[:, ct0 * 128:(ct0 + C) * 128])
                    ct0 += C
                    V = e1s.tile([128, C, 264], BF16, tag="V")

                    for w in ws:
                        cols = [t for a0, a1 in spans[w] for t in range(a0, a1)]
                        Dwp = e1tp.tile([128, len(cols), 16], FP32, tag="dv")
                        for j, t in enumerate(cols):
                            nc.tensor.matmul(Dwp[:, j, :], lhsT=SttC[:, t, :],
                                             rhs=dtab1[:, w, :], start=True, stop=True)
                        j0 = 0
                        for a0, a1 in spans[w]:
                            n = a1 - a0
                            if n == 0:
                                continue
                            A = e1t.tile([128, n, 16], BF16, tag="A")
                            nc.vector.tensor_tensor(
                                out=A[:], in0=Gt[:, a0:a1, 256:288].bitcast(BF16),
                                in1=Dwp[:, j0:j0 + n, :], op=OP.mult)
                            nc.vector.tensor_tensor(out=V[:, a0:a1, 256:264],
                                                    in0=A[:, :, 0:8], in1=A[:, :, 8:16],
                                                    op=OP.max)
                            nc.vector.tensor_tensor(
                                out=V[:, a0:a1, 0:256].rearrange("p c (h d) -> p c h d", h=8),
                                in0=Gt[:, a0:a1, 0:256].bitcast(FP8).rearrange(
                                    "p c (h d) -> p c h d", h=8),
                                in1=V[:, a0:a1, 256:264][:, :, :, None].to_broadcast([128, n, 8, 32]),
                                op=OP.mult)
                            j0 += n
                        psw = e1ps.tile([128, 264], FP32, tag="psw")
                        for j, t in enumerate(cols):
                            nc.tensor.matmul(psw[:], lhsT=St[:, t, :], rhs=V[:, t, :],
                                             start=(j == 0), stop=(j == len(cols) - 1))
                        den = ep.tile([128, 8], FP32, tag="den")
                        nc.vector.tensor_scalar_max(den[:], psw[:, 256:264], 1e-30)
                        rden = ep.tile([128, 8], FP32, tag="rden")
                        nc.vector.reciprocal(rden[:], den[:])
                        z = ep.tile([128, 256], FP32, tag="z")
                        nc.vector.tensor_tensor(
                            out=z[:].rearrange("p (h d) -> p h d", h=8),
                            in0=psw[:, 0:256].rearrange("p (h d) -> p h d", h=8),
                            in1=rden[:][:, :, None].to_broadcast([128, 8, 32]),
                            op=OP.mult)
                        nc.vector.tensor_tensor(out=z[:], in0=z[:], in1=b1sb[:],
                                                op=OP.add)
                        r = ep.tile([128, 256], FP32, tag="r")
                        nc.scalar.activation(r[:], z[:], AF.Relu)
                        t1 = ep.tile([128, 256], FP32, tag="t1")
                        nc.scalar.activation(t1[:], z[:], AF.Relu, scale=-1.0)
                        e_ = ep.tile([128, 256], FP32, tag="e_")
                        nc.scalar.activation(e_[:], t1[:], AF.Exp, scale=-1.0)
                        h2p = ep.tile([128, 256], BF16, tag="h2p")
                        nc.vector.tensor_tensor(out=h2p[:], in0=r[:], in1=e_[:],
                                                op=OP.add)
                        h2pT = ep.tile([128, 2, 128], BF16, tag="h2pT")
                        for k in range(2):
                            ptr2 = e1tp.tile([128, 128], BF16, tag="ptr")
                            nc.tensor.transpose(ptr2[:], h2p[:, ts(k, 128)], idb[:])
                            if k == 0:
                                nc.vector.tensor_copy(out=h2pT[:, k, :], in_=ptr2[:])
                            else:
                                nc.scalar.copy(out=h2pT[:, k, :], in_=ptr2[:])
                        ps2 = e1ps.tile([128, 34], FP32, tag="ps2")
                        nc.tensor.matmul(ps2[:], lhsT=h2pT[:, 0, :], rhs=w2sb[:, 0, :],
                                         start=True, stop=False)
                        nc.tensor.matmul(ps2[:], lhsT=h2pT[:, 1, :], rhs=w2sb[:, 1, :],
                                         start=False, stop=True)
                        h2row = ep.tile([128, 128], BF16, tag="h2row")
                        nc.vector.tensor_tensor(out=h2row[:, 0:32], in0=ps2[:, 0:32],
                                                in1=c2sb[:], op=OP.subtract)
                        nc.scalar.activation(h2row[:, 32:33], ps2[:, 32:33], AF.Exp,
                                             bias=bexp[:, 0:1])
                        nc.scalar.activation(h2row[:, 33:34], ps2[:, 32:33], AF.Exp,
                                             scale=0.2, bias=bexp[:, 1:2])
                        nc.scalar.activation(dtab2[:, w, 0:1], ps2[:, 33:34], AF.Exp,
                                             bias=bexp[:, 2:3])
                        nc.scalar.activation(dtab2[:, w, 1:2], ps2[:, 33:34], AF.Exp,
                                             scale=0.2, bias=bexp[:, 3:4])
                        nc.sync.dma_start(out=h2x_mine[ts(w, 128), :], in_=h2row[:])

            nc.gpsimd.collective_compute(
                "AllGather", OP.bypass, replica_groups=rg,
                ins=[h2x_mine[:]], outs=[h2x_full[:]])

            # ---------------- Phase E2: layer-2 edges + pooling ----------------
            with tc.tile_pool(name="e2", bufs=3) as e2, \
                 tc.tile_pool(name="e2s", bufs=2) as e2s, \
                 tc.tile_pool(name="e2t", bufs=3) as e2t, \
                 tc.tile_pool(name="e2ep", bufs=2) as ep2, \
                 tc.tile_pool(name="e2tp", bufs=2, space="PSUM") as e2tp, \
                 tc.tile_pool(name="e2ps", bufs=2, space="PSUM") as e2ps, \
                 tc.tile_pool(name="poolps", bufs=1, space="PSUM") as plps:
                pspool = plps.tile([32, 33], FP32)
                olo = ohi = 0
                ct0 = 0
                wcount = 0
                for s, ws in enumerate(chunks):
                    spans, C = _chunk_cols(T_LO, T_HI, ws)
                    ntl = sum(T_LO[w] for w in ws)
                    nlo, nhi = ntl * 128, (C - ntl) * 128
                    G2 = e2.tile([128, C, 128], BF16, tag="G2")
                    if nlo:
                        ilo = e2.tile([128, nlo // 16], I16, tag="ilo2")
                        nc.sync.dma_start(out=ilo[:], in_=idxlo[:, olo:olo + nlo // 16])
                        nc.gpsimd.dma_gather(
                            out_ap=G2[:, 0:ntl, :], in_ap=h2x_full[0:HALF, :],
                            idxs_ap=ilo[:], num_idxs=nlo, num_idxs_reg=nlo,
                            elem_size=128, single_packet=False, queue_num=0)
                    if nhi:
                        ihi = e2.tile([128, nhi // 16], I16, tag="ihi2")
                        nc.sync.dma_start(out=ihi[:], in_=idxhi[:, ohi:ohi + nhi // 16])
                        nc.gpsimd.dma_gather(
                            out_ap=G2[:, ntl:C, :], in_ap=h2x_full[HALF:NFULL, :],
                            idxs_ap=ihi[:], num_idxs=nhi, num_idxs_reg=nhi,
                            elem_size=128, single_packet=False, queue_num=1)
                    olo += nlo // 16; ohi += nhi // 16

                    S2 = e2s.tile([128, C, 128], BF16, tag="S2")
                    nc.vector.tensor_tensor(
                        out=S2[:],
                        in0=iota_bf[:][:, None, :].to_broadcast([128, C, 128]),
                        in1=drel_sb[:, ct0:ct0 + C][:, :, None].to_broadcast([128, C, 128]),
                        op=OP.is_equal)
                    SttC = e2s.tile([128, C, 128], BF16, tag="SttC2")
                    nc.sync.dma_start(out=SttC[:],
                                      in_=sttoh[:, ct0 * 128:(ct0 + C) * 128])
                    ct0 += C
                    V2 = e2s.tile([128, C, 33], BF16, tag="V2")

                    for w in ws:
                        cols = [t for a0, a1 in spans[w] for t in range(a0, a1)]
                        Dwp2 = e2tp.tile([128, len(cols), 2], FP32, tag="dv")
                        for j, t in enumerate(cols):
                            nc.tensor.matmul(Dwp2[:, j, :], lhsT=SttC[:, t, :],
                                             rhs=dtab2[:, w, :], start=True, stop=True)
                        j0 = 0
                        for a0, a1 in spans[w]:
                            n = a1 - a0
                            if n == 0:
                                continue
                            A2 = e2t.tile([128, n, 2], BF16, tag="A2")
                            nc.vector.tensor_tensor(out=A2[:], in0=G2[:, a0:a1, 32:34],
                                                    in1=Dwp2[:, j0:j0 + n, :], op=OP.mult)
                            nc.vector.tensor_tensor(out=V2[:, a0:a1, 32:33],
                                                    in0=A2[:, :, 0:1], in1=A2[:, :, 1:2],
                                                    op=OP.max)
                            nc.vector.tensor_tensor(
                                out=V2[:, a0:a1, 0:32], in0=G2[:, a0:a1, 0:32],
                                in1=V2[:, a0:a1, 32:33].to_broadcast([128, n, 32]),
                                op=OP.mult)
                            j0 += n
                        psw2 = e2ps.tile([128, 33], FP32, tag="psw2")
                        for j, t in enumerate(cols):
                            nc.tensor.matmul(psw2[:], lhsT=S2[:, t, :], rhs=V2[:, t, :],
                                             start=(j == 0), stop=(j == len(cols) - 1))
                        den2 = ep2.tile([128, 1], FP32, tag="den2")
                        nc.vector.tensor_scalar_max(den2[:], psw2[:, 32:33], 1e-30)
                        rd2 = ep2.tile([128, 1], FP32, tag="rd2")
                        nc.vector.reciprocal(rd2[:], den2[:])
                        z2 = ep2.tile([128, 32], FP32, tag="z2")
                        nc.vector.tensor_scalar(out=z2[:], in0=psw2[:, 0:32],
                                                scalar1=rd2[:, 0:1], scalar2=None,
                                                op0=OP.mult)
                        nc.vector.tensor_tensor(out=z2[:], in0=z2[:], in1=b2sb[:],
                                                op=OP.add)
                        r2 = ep2.tile([128, 32], FP32, tag="r2")
                        nc.scalar.activation(r2[:], z2[:], AF.Relu)
                        t2 = ep2.tile([128, 32], FP32, tag="t2")
                        nc.scalar.activation(t2[:], z2[:], AF.Relu, scale=-1.0)
                        e2_ = ep2.tile([128, 32], FP32, tag="e2_")
                        nc.scalar.activation(e2_[:], t2[:], AF.Exp, scale=-1.0)
                        h3a = ep2.tile([128, 33], FP32, tag="h3a")
                        nc.vector.tensor_tensor(out=h3a[:, 0:32], in0=r2[:],
                                                in1=e2_[:], op=OP.add)
                        nc.vector.memset(h3a[:, 32:33], 1.0)
                        sbt = ep2.tile([128, 32], FP32, tag="sbt")
                        nc.sync.dma_start(out=sbt[:], in_=Sb[ts(w, 128), :])
                        nc.tensor.matmul(pspool[:], lhsT=sbt[:], rhs=h3a[:],
                                         start=(wcount == 0), stop=(wcount == NW - 1))
                        wcount += 1

                # ---------------- Phase F: AllReduce + MLP ----------------
                poolsb = ep2.tile([32, 33], FP32)
                nc.vector.tensor_copy(out=poolsb[:], in_=pspool[:])
                nc.sync.dma_start(out=pool_mine[:], in_=poolsb[:])
                nc.gpsimd.collective_compute(
                    "AllReduce", OP.add, replica_groups=rg,
                    ins=[pool_mine[:]], outs=[pool_sum[:]])
                psf = ep2.tile([32, 33], FP32)
                nc.sync.dma_start(out=psf[:], in_=pool_sum[:])
                cntc = ep2.tile([32, 1], FP32)
                nc.vector.tensor_scalar_max(cntc[:], psf[:, 32:33], 1.0)
                rc = ep2.tile([32, 1], FP32)
                nc.vector.reciprocal(rc[:], cntc[:])
                gv = ep2.tile([32, 32], FP32)
                nc.vector.tensor_scalar(out=gv[:], in0=psf[:, 0:32],
                                        scalar1=rc[:, 0:1], scalar2=-1.0,
                                        op0=OP.mult, op1=OP.add)
                ptg = e2ps.tile([32, 32], FP32, tag="mlp", bufs=1)
                nc.tensor.transpose(ptg[:], gv[:], idf[0:32, 0:32])
                gvT = ep2.tile([32, 32], FP32)
                nc.vector.tensor_copy(out=gvT[:], in_=ptg[:])
                psh = e2ps.tile([16, 32], FP32, tag="mlp", bufs=1)
                nc.tensor.matmul(psh[:], lhsT=wc1sb[:], rhs=gvT[:],
                                 start=True, stop=True)
                hidT = ep2.tile([16, 32], FP32)
                nc.scalar.activation(hidT[:], psh[:], AF.Relu, bias=bc1sb[:])
                psr = e2ps.tile([1, 32], FP32, tag="mlp", bufs=1)
                nc.tensor.matmul(psr[:], lhsT=wc2sb[:], rhs=hidT[:],
                                 start=True, stop=True)
                rsb = ep2.tile([1, 32], FP32)
                nc.scalar.activation(rsb[:], psr[:], AF.Copy, bias=bc2f)
                nc.sync.dma_start(out=riskT[:], in_=rsb[:])

    nc.compile()
    return nc


def kernel(**inputs):
    global last_results
    x = np.asarray(inputs["x"], np.float32)
    ei = np.asarray(inputs["edge_index"])
    batch = np.asarray(inputs["batch"]).astype(np.int64)
    W1 = np.asarray(inputs["W1"], np.float32)
    as1 = np.asarray(inputs["att_src1"], np.float32)
    ad1 = np.asarray(inputs["att_dst1"], np.float32)
    b1 = np.asarray(inputs["b1"], np.float32)
    W2 = np.asarray(inputs["W2"], np.float32)
    as2 = np.asarray(inputs["att_src2"], np.float32)
    ad2 = np.asarray(inputs["att_dst2"], np.float32)
    b2 = np.asarray(inputs["b2"], np.float32)
    Wc1 = np.asarray(inputs["Wc1"], np.float32)
    bc1 = np.asarray(inputs["bc1"], np.float32)
    Wc2 = np.asarray(inputs["Wc2"], np.float32)
    bc2 = np.asarray(inputs["bc2"], np.float32)

    T_LO, T_HI, chunks, percore, node_core, node_local = _prep_edges(ei)
    CT = int((T_LO + T_HI).sum())
    LOCOLS = int(T_LO.sum()) * 8
    HICOLS = int(T_HI.sum()) * 8

    A_s = np.zeros((256, 8), np.float32)
    A_d = np.zeros((256, 8), np.float32)
    for h in range(H):
        A_s[h * 32:(h + 1) * 32, h] = as1[h]
        A_d[h * 32:(h + 1) * 32, h] = ad1[h]
    W1ext = np.hstack([W1, W1 @ A_s, W1 @ A_d]).astype(bf16)
    W2ext = np.hstack([W2, W2 @ as2[0][:, None], W2 @ ad2[0][:, None]]).astype(bf16)
    c2 = np.ones(256, np.float32) @ W2ext.astype(np.float32)  # [34]

    nc = _build_program(T_LO, T_HI, chunks, CT, LOCOLS, HICOLS,
                        float(c2[32]), float(c2[33]), float(bc2.ravel()[0]))

    in_maps = []
    for c in range(NCORES):
        lo, hi, dr = percore[c]
        mine = node_core == c
        locs = node_local[mine]
        xs = np.zeros((256, NLOCP), bf16)
        xs[:, locs] = x[mine].T.astype(bf16)
        Sbm = np.zeros((NLOCP, 32), np.float32)
        Sbm[locs, batch[mine]] = 1.0
        dr2 = dr.reshape(CT, 128)
        oh = (dr2[:, None, :] == np.arange(128)[None, :, None])
        sttv = np.ascontiguousarray(
            oh.transpose(1, 0, 2).reshape(128, CT * 128)).astype(bf16)
        in_maps.append({
            "xT": np.ascontiguousarray(xs),
            "idxlo": _wrap_idx(lo), "idxhi": _wrap_idx(hi),
            "dstrel": np.ascontiguousarray(dr.reshape(CT, 128).T.astype(bf16)),
            "sttoh": sttv,
            "Sb": Sbm,
            "W1e": W1ext, "W2e": W2ext,
            "b1r": np.ascontiguousarray(np.broadcast_to(b1, (128, 256))).astype(np.float32),
            "b2r": np.ascontiguousarray(np.broadcast_to(b2, (128, 32))).astype(np.float32),
            "c2r": np.ascontiguousarray(np.broadcast_to(c2[0:32], (128, 32))).astype(np.float32),
            "bc1t": bc1.reshape(16, 1).astype(np.float32),
            "Wc1t": Wc1, "Wc2t": Wc2.reshape(16, 1),
        })
    res = run_bass_kernel_spmd(nc, in_maps, core_ids=list(range(NCORES)))
    last_results = res
    return res.results[0]["riskT"].reshape(32, 1).astype(np.float32)


# revision 19
# speedup vs baseline: 1.1697x; 1.1697x over previous
"""Trainium2 Bass kernel for nn_GATSurvival (2-layer GAT + mean-pool + MLP).

Strategy (8 NeuronCores, SPMD):
- Nodes assigned to cores by balanced in-degree (snake deal over degree-sorted
  nodes) so per-core edge counts match; dst windows of 128 local nodes.
- Projection sharded: each core computes h1 = x_slab @ [W1 | W1@As | W1@Ad],
  writes 768B/row bf16 table [h1(256) | u=exp(as)(8) | u2=exp(.2as)(8) | pad],
  AllGathers the full table. dst-side exp(ad)/exp(.2ad) go straight into an
  SBUF-resident table (no DRAM roundtrip, no dst gather).
- Per-edge logits use exp(leaky(as+ad)) == max(exp(as)exp(ad), exp(.2as)exp(.2ad)).
- Only the src rows are dma_gathered (int16 idx, lo/hi table halves, two SWDGE
  queues alternated to pipeline descriptor generation against drain).
- Per-edge dst values come from a one-hot select matmul: St (edge x dst one-hot,
  built by is_equal) is transposed on the tensor engine, then StT.T @ dtab_win
  yields v per edge. Scatter-softmax/aggregation per 128-dst window via
  psum += St.T @ [alpha*h | alpha]; divide by the alpha-sum.
- Layer 2 mirrors layer 1 with 256B rows; graph mean-pool via a batch one-hot
  matmul accumulated in PSUM, AllReduce, and a tiny on-device MLP.
"""

import numpy as np
import ml_dtypes

import concourse.bass as bass
import concourse.mybir as mybir
from concourse import bacc
from concourse.tile import TileContext
from concourse.bass import ts
from concourse.bass_utils import run_bass_kernel_spmd
from concourse.masks import make_identity

bf16 = ml_dtypes.bfloat16
FP32 = mybir.dt.float32
BF16 = mybir.dt.bfloat16
I16 = mybir.dt.int16
U8 = mybir.dt.uint8
FP8 = mybir.dt.float8e4
AF = mybir.ActivationFunctionType
OP = mybir.AluOpType

NCORES, NLOCP = 8, 6272
NW = 49
SPLIT_W = 25          # windows in AllGather piece A
LOCA = SPLIT_W * 128  # 3200 rows/core in piece A
LOCB = NLOCP - LOCA   # 3072
HALF = NCORES * LOCA  # 25600: lo/hi gather-table boundary
NFULL = 8 * NLOCP     # 50176
N, F, H, D = 50000, 256, 8, 32
G = 32
TCAP = 20             # max gather cols per chunk-half (ring: 8*TCAP+1 descs/engine)

last_results = None  # BassKernelResults of the most recent run (for test harness)


def _balance_nodes(dst_all):
    """LPT-pack nodes into the 8*NW (core, window) buckets balancing in-degree.

    Returns (node_core, node_local)."""
    import heapq
    deg = np.bincount(dst_all, minlength=N)
    order = np.argsort(-deg, kind="stable")
    nbuckets = NCORES * NW
    heap = [(0, b) for b in range(nbuckets)]
    heapq.heapify(heap)
    fill = np.zeros(nbuckets, np.int64)
    node_core = np.empty(N, np.int64)
    node_local = np.empty(N, np.int64)
    for n in order:
        load, b = heapq.heappop(heap)
        node_core[n] = b // NW
        node_local[n] = (b % NW) * 128 + fill[b]
        fill[b] += 1
        if fill[b] < 128:
            heapq.heappush(heap, (load + int(deg[n]), b))
    return node_core, node_local


def _prep_edges(edge_index):
    src = np.concatenate([np.asarray(edge_index[0]), np.arange(N)]).astype(np.int64)
    dst = np.concatenate([np.asarray(edge_index[1]), np.arange(N)]).astype(np.int64)
    node_core, node_local = _balance_nodes(dst)
    core = node_core[dst]
    dst_local = node_local[dst]
    src_loc = node_local[src]
    table_row = np.where(src_loc < LOCA,
                         node_core[src] * LOCA + src_loc,
                         HALF + node_core[src] * LOCB + (src_loc - LOCA))
    window = dst_local >> 7
    dst_rel = dst_local & 127
    is_hi = (table_row >= HALF).astype(np.int64)
    order = np.lexsort((is_hi, window, core))
    tr_s, drel_s = table_row[order], dst_rel[order]
    key = (core[order] * NW + window[order]) * 2 + is_hi[order]
    counts = np.bincount(key, minlength=NCORES * NW * 2).reshape(NCORES, NW, 2)
    T_LO = -(-counts[:, :, 0].max(axis=0) // 128)
    T_HI = -(-counts[:, :, 1].max(axis=0) // 128)
    # descriptor ring holds ~256 descs/engine; one gather needs 8*cols+1
    assert T_LO.max() <= 24 and T_HI.max() <= 24, (T_LO.max(), T_HI.max())
    # per-window select results go in one PSUM bank: Tw*16*4B <= 2KB
    assert (T_LO + T_HI).max() <= 32, (T_LO + T_HI).max()
    # greedy chunking: pack windows while both halves stay under TCAP columns
    chunks, cur, lo_sum, hi_sum = [], [], 0, 0
    for w in range(NW):
        if cur and (lo_sum + T_LO[w] > TCAP or hi_sum + T_HI[w] > TCAP):
            chunks.append(tuple(cur))
            cur, lo_sum, hi_sum = [], 0, 0
        cur.append(w)
        lo_sum += T_LO[w]
        hi_sum += T_HI[w]
    if cur:
        chunks.append(tuple(cur))
    starts = np.zeros(NCORES * NW * 2 + 1, dtype=np.int64)
    np.cumsum(counts.reshape(-1), out=starts[1:])
    percore = []
    for c in range(NCORES):
        lo_vals, hi_vals, dr_vals = [], [], []
        for ws in chunks:
            for w in ws:  # lo blocks first
                k = (c * NW + w) * 2
                ev = tr_s[starts[k]:starts[k + 1]]
                pad = T_LO[w] * 128 - len(ev)
                lo_vals.append(np.concatenate([ev, np.zeros(pad, np.int64)]))
                dr_vals.append(np.concatenate([drel_s[starts[k]:starts[k + 1]],
                                               np.full(pad, 255, np.int64)]))
            for w in ws:  # then hi blocks
                k = (c * NW + w) * 2 + 1
                ev = tr_s[starts[k]:starts[k + 1]] - HALF
                pad = T_HI[w] * 128 - len(ev)
                hi_vals.append(np.concatenate([ev, np.zeros(pad, np.int64)]))
                dr_vals.append(np.concatenate([drel_s[starts[k]:starts[k + 1]],
                                               np.full(pad, 255, np.int64)]))
        percore.append((np.concatenate(lo_vals), np.concatenate(hi_vals),
                        np.concatenate(dr_vals)))
    return T_LO, T_HI, chunks, percore, node_core, node_local


def _wrap_idx(vals):
    """[n] ints -> [128, n//16] int16 (16-partition wrap, replicated 8x)."""
    n = len(vals)
    w = vals.reshape(n // 16, 16).T.astype(np.int16)
    return np.ascontiguousarray(np.tile(w, (8, 1)))


def _chunk_cols(T_LO, T_HI, ws):
    """Per-window (lo_range, hi_range) column spans within a chunk."""
    spans = {}
    base = 0
    for w in ws:
        spans[w] = [(base, base + T_LO[w])]
        base += T_LO[w]
    for w in ws:
        spans[w].append((base, base + T_HI[w]))
        base += T_HI[w]
    return spans, base


def _build_program(T_LO, T_HI, chunks, CT, LOCOLS, HICOLS, c2_32, c2_33, bc2f):
    nc = bacc.Bacc(num_devices=NCORES, num_swdge_queues=2)

    xT = nc.dram_tensor("xT", [256, NLOCP], BF16, kind="ExternalInput")
    idxlo = nc.dram_tensor("idxlo", [128, LOCOLS], I16, kind="ExternalInput")
    idxhi = nc.dram_tensor("idxhi", [128, HICOLS], I16, kind="ExternalInput")
    dstrel = nc.dram_tensor("dstrel", [128, CT], BF16, kind="ExternalInput")
    sttoh = nc.dram_tensor("sttoh", [128, CT * 128], BF16, kind="ExternalInput")
    Sb = nc.dram_tensor("Sb", [NLOCP, 32], FP32, kind="ExternalInput")
    W1e = nc.dram_tensor("W1e", [256, 272], BF16, kind="ExternalInput")
    W2e = nc.dram_tensor("W2e", [256, 34], BF16, kind="ExternalInput")
    b1r = nc.dram_tensor("b1r", [128, 256], FP32, kind="ExternalInput")
    b2r = nc.dram_tensor("b2r", [128, 32], FP32, kind="ExternalInput")
    c2r = nc.dram_tensor("c2r", [128, 32], FP32, kind="ExternalInput")
    bc1t = nc.dram_tensor("bc1t", [16, 1], FP32, kind="ExternalInput")
    Wc1t = nc.dram_tensor("Wc1t", [32, 16], FP32, kind="ExternalInput")
    Wc2t = nc.dram_tensor("Wc2t", [16, 1], FP32, kind="ExternalInput")
    riskT = nc.dram_tensor("riskT", [1, 32], FP32, kind="ExternalOutput")

    h1x_mine = nc.dram_tensor("h1x_mine", [NLOCP, 512], U8, kind="Internal")
    h1x_full = nc.dram_tensor("h1x_full", [NFULL, 512], U8, kind="Internal",
                              addr_space="Shared")
    h2x_mine = nc.dram_tensor("h2x_mine", [NLOCP, 128], BF16, kind="Internal")
    h2x_full = nc.dram_tensor("h2x_full", [NFULL, 128], BF16, kind="Internal",
                              addr_space="Shared")
    pool_mine = nc.dram_tensor("pool_mine", [32, 33], FP32, kind="Internal")
    pool_sum = nc.dram_tensor("pool_sum", [32, 33], FP32, kind="Internal",
                              addr_space="Shared")

    rg = [list(range(NCORES))]

    with TileContext(nc) as tc:
        with tc.tile_pool(name="const", bufs=1) as cp:
            w1sb = cp.tile([128, 2, 272], BF16)
            nc.sync.dma_start(out=w1sb[:, 0, :], in_=W1e[0:128, :])
            nc.sync.dma_start(out=w1sb[:, 1, :], in_=W1e[128:256, :])
            w2sb = cp.tile([128, 2, 34], BF16)
            nc.sync.dma_start(out=w2sb[:, 0, :], in_=W2e[0:128, :])
            nc.sync.dma_start(out=w2sb[:, 1, :], in_=W2e[128:256, :])
            b1sb = cp.tile([128, 256], FP32)
            nc.sync.dma_start(out=b1sb[:], in_=b1r[:])
            b2sb = cp.tile([128, 32], FP32)
            nc.sync.dma_start(out=b2sb[:], in_=b2r[:])
            c2sb = cp.tile([128, 32], FP32)
            nc.sync.dma_start(out=c2sb[:], in_=c2r[:])
            bc1sb = cp.tile([16, 1], FP32)
            nc.sync.dma_start(out=bc1sb[:], in_=bc1t[:])
            wc1sb = cp.tile([32, 16], FP32)
            nc.sync.dma_start(out=wc1sb[:], in_=Wc1t[:])
            wc2sb = cp.tile([16, 1], FP32)
            nc.sync.dma_start(out=wc2sb[:], in_=Wc2t[:])
            drel_sb = cp.tile([128, CT], BF16)
            nc.sync.dma_start(out=drel_sb[:], in_=dstrel[:])
            iota_i = cp.tile([128, 128], mybir.dt.int32)
            nc.gpsimd.iota(iota_i[:], pattern=[[1, 128]], base=0, channel_multiplier=0)
            iota_bf = cp.tile([128, 128], BF16)
            nc.vector.tensor_copy(out=iota_bf[:], in_=iota_i[:])
            idf = cp.tile([128, 128], FP32)
            make_identity(nc, idf[:])
            idb = cp.tile([128, 128], BF16)
            nc.vector.tensor_copy(out=idb[:], in_=idf[:])
            bexp = cp.tile([128, 4], FP32)
            nc.vector.memset(bexp[:, 0:1], -c2_32)
            nc.vector.memset(bexp[:, 1:2], -0.2 * c2_32)
            nc.vector.memset(bexp[:, 2:3], -c2_33)
            nc.vector.memset(bexp[:, 3:4], -0.2 * c2_33)
            # SBUF-resident per-dst attention tables (written in P / E1 epilogue)
            dtab1 = cp.tile([128, NW, 16], BF16)
            dtab2 = cp.tile([128, NW, 2], BF16)

            # ---------------- Phase P: projection ----------------
            with tc.tile_pool(name="proj", bufs=3) as pp, \
                 tc.tile_pool(name="projps", bufs=2, space="PSUM") as ppp, \
                 tc.tile_pool(name="xs", bufs=3) as xp:
                for nt in range(NW):
                    xs = xp.tile([128, 2, 128], BF16, tag="xs")
                    nc.sync.dma_start(out=xs[:, 0, :], in_=xT[0:128, ts(nt, 128)])
                    nc.sync.dma_start(out=xs[:, 1, :], in_=xT[128:256, ts(nt, 128)])
                    ps = ppp.tile([128, 272], FP32, tag="ps")
                    nc.tensor.matmul(ps[:], lhsT=xs[:, 0, :],
                                     rhs=w1sb[:, 0, :], start=True, stop=False)
                    nc.tensor.matmul(ps[:], lhsT=xs[:, 1, :],
                                     rhs=w1sb[:, 1, :], start=False, stop=True)
                    hrow = pp.tile([128, 512], U8, tag="hrow")
                    nc.vector.tensor_copy(out=hrow[:, 0:256].bitcast(FP8),
                                          in_=ps[:, 0:256])
                    nc.scalar.activation(hrow[:, 256:272].bitcast(BF16),
                                         ps[:, 256:264], AF.Exp)
                    nc.scalar.activation(hrow[:, 272:288].bitcast(BF16),
                                         ps[:, 256:264], AF.Exp, scale=0.2)
                    nc.scalar.activation(dtab1[:, nt, 0:8], ps[:, 264:272], AF.Exp)
                    nc.scalar.activation(dtab1[:, nt, 8:16], ps[:, 264:272], AF.Exp,
                                         scale=0.2)
                    nc.sync.dma_start(out=h1x_mine[ts(nt, 128), :], in_=hrow[:])
                    if nt == SPLIT_W - 1:
                        nc.gpsimd.collective_compute(
                            "AllGather", OP.bypass, replica_groups=rg,
                            ins=[h1x_mine[0:LOCA, :]], outs=[h1x_full[0:HALF, :]])

            nc.gpsimd.collective_compute(
                "AllGather", OP.bypass, replica_groups=rg,
                ins=[h1x_mine[LOCA:NLOCP, :]], outs=[h1x_full[HALF:NFULL, :]])

            # ---------------- Phase E1: layer-1 edges ----------------
            with tc.tile_pool(name="e1", bufs=2) as e1, \
                 tc.tile_pool(name="e1s", bufs=2) as e1s, \
                 tc.tile_pool(name="e1t", bufs=3) as e1t, \
                 tc.tile_pool(name="e1ep", bufs=2) as ep, \
                 tc.tile_pool(name="e1tp", bufs=2, space="PSUM") as e1tp, \
                 tc.tile_pool(name="e1ps", bufs=2, space="PSUM") as e1ps:
                olo = ohi = 0
                ct0 = 0
                for s, ws in enumerate(chunks):
                    spans, C = _chunk_cols(T_LO, T_HI, ws)
                    ntl = sum(T_LO[w] for w in ws)
                    nlo, nhi = ntl * 128, (C - ntl) * 128
                    Gt = e1.tile([128, C, 512], U8, tag="G")
                    if nlo:
                        ilo = e1.tile([128, nlo // 16], I16, tag="ilo")
                        nc.sync.dma_start(out=ilo[:], in_=idxlo[:, olo:olo + nlo // 16])
                        nc.gpsimd.dma_gather(
                            out_ap=Gt[:, 0:ntl, :], in_ap=h1x_full[0:HALF, :],
                            idxs_ap=ilo[:], num_idxs=nlo, num_idxs_reg=nlo,
                            elem_size=512, single_packet=False, queue_num=0)
                    if nhi:
                        ihi = e1.tile([128, nhi // 16], I16, tag="ihi")
                        nc.sync.dma_start(out=ihi[:], in_=idxhi[:, ohi:ohi + nhi // 16])
                        nc.gpsimd.dma_gather(
                            out_ap=Gt[:, ntl:C, :], in_ap=h1x_full[HALF:NFULL, :],
                            idxs_ap=ihi[:], num_idxs=nhi, num_idxs_reg=nhi,
                            elem_size=512, single_packet=False, queue_num=1)
                    olo += nlo // 16; ohi += nhi // 16

                    St = e1s.tile([128, C, 128], BF16, tag="St")
                    nc.vector.tensor_tensor(
                        out=St[:],
                        in0=iota_bf[:][:, None, :].to_broadcast([128, C, 128]),
                        in1=drel_sb[:, ct0:ct0 + C][:, :, None].to_broadcast([128, C, 128]),
                        op=OP.is_equal)
                    SttC = e1s.tile([128, C, 128], BF16, tag="SttC")
                    nc.sync.dma_start(out=SttC[:],
                                      in_=sttoh[:, ct0 * 128:(ct0 + C) * 128])
                    ct0 += C
                    V = e1s.tile([128, C, 264], BF16, tag="V")

                    for w in ws:
                        cols = [t for a0, a1 in spans[w] for t in range(a0, a1)]
                        Dwp = e1tp.tile([128, len(cols), 16], FP32, tag="dv")
                        for j, t in enumerate(cols):
                            nc.tensor.matmul(Dwp[:, j, :], lhsT=SttC[:, t, :],
                                             rhs=dtab1[:, w, :], start=True, stop=True)
                        j0 = 0
                        for a0, a1 in spans[w]:
                            n = a1 - a0
                            if n == 0:
                                continue
                            A = e1t.tile([128, n, 16], BF16, tag="A")
                            nc.vector.tensor_tensor(
                                out=A[:], in0=Gt[:, a0:a1, 256:288].bitcast(BF16),
                                in1=Dwp[:, j0:j0 + n, :], op=OP.mult)
                            nc.vector.tensor_tensor(out=V[:, a0:a1, 256:264],
                                                    in0=A[:, :, 0:8], in1=A[:, :, 8:16],
                                                    op=OP.max)
                            nc.vector.tensor_tensor(
                                out=V[:, a0:a1, 0:256].rearrange("p c (h d) -> p c h d", h=8),
                                in0=Gt[:, a0:a1, 0:256].bitcast(FP8).rearrange(
                                    "p c (h d) -> p c h d", h=8),
                                in1=V[:, a0:a1, 256:264][:, :, :, None].to_broadcast([128, n, 8, 32]),
                                op=OP.mult)
                            j0 += n
                        psw = e1ps.tile([128, 264], FP32, tag="psw")
                        for j, t in enumerate(cols):
                            nc.tensor.matmul(psw[:], lhsT=St[:, t, :], rhs=V[:, t, :],
                                             start=(j == 0), stop=(j == len(cols) - 1))
                        den = ep.tile([128, 8], FP32, tag="den")
                        nc.vector.tensor_scalar_max(den[:], psw[:, 256:264], 1e-30)
                        rden = ep.tile([128, 8], FP32, tag="rden")
                        nc.vector.reciprocal(rden[:], den[:])
                        z = ep.tile([128, 256], FP32, tag="z")
                        nc.vector.tensor_tensor(
                            out=z[:].rearrange("p (h d) -> p h d", h=8),
                            in0=psw[:, 0:256].rearrange("p (h d) -> p h d", h=8),
                            in1=rden[:][:, :, None].to_broadcast([128, 8, 32]),
                            op=OP.mult)
                        nc.vector.tensor_tensor(out=z[:], in0=z[:], in1=b1sb[:],
                                                op=OP.add)
                        r = ep.tile([128, 256], FP32, tag="r")
                        nc.scalar.activation(r[:], z[:], AF.Relu)
                        t1 = ep.tile([128, 256], FP32, tag="t1")
                        nc.scalar.activation(t1[:], z[:], AF.Relu, scale=-1.0)
                        e_ = ep.tile([128, 256], FP32, tag="e_")
                        nc.scalar.activation(e_[:], t1[:], AF.Exp, scale=-1.0)
                        h2p = ep.tile([128, 256], BF16, tag="h2p")
                        nc.vector.tensor_tensor(out=h2p[:], in0=r[:], in1=e_[:],
                                                op=OP.add)
                        h2pT = ep.tile([128, 2, 128], BF16, tag="h2pT")
                        for k in range(2):
                            ptr2 = e1tp.tile([128, 128], BF16, tag="ptr")
                            nc.tensor.transpose(ptr2[:], h2p[:, ts(k, 128)], idb[:])
                            if k == 0:
                                nc.vector.tensor_copy(out=h2pT[:, k, :], in_=ptr2[:])
                            else:
                                nc.scalar.copy(out=h2pT[:, k, :], in_=ptr2[:])
                        ps2 = e1ps.tile([128, 34], FP32, tag="ps2")
                        nc.tensor.matmul(ps2[:], lhsT=h2pT[:, 0, :], rhs=w2sb[:, 0, :],
                                         start=True, stop=False)
                        nc.tensor.matmul(ps2[:], lhsT=h2pT[:, 1, :], rhs=w2sb[:, 1, :],
                                         start=False, stop=True)
                        h2row = ep.tile([128, 128], BF16, tag="h2row")
                        nc.vector.tensor_tensor(out=h2row[:, 0:32], in0=ps2[:, 0:32],
                                                in1=c2sb[:], op=OP.subtract)
                        nc.scalar.activation(h2row[:, 32:33], ps2[:, 32:33], AF.Exp,
                                             bias=bexp[:, 0:1])
                        nc.scalar.activation(h2row[:, 33:34], ps2[:, 32:33], AF.Exp,
                                             scale=0.2, bias=bexp[:, 1:2])
                        nc.scalar.activation(dtab2[:, w, 0:1], ps2[:, 33:34], AF.Exp,
                                             bias=bexp[:, 2:3])
                        nc.scalar.activation(dtab2[:, w, 1:2], ps2[:, 33:34], AF.Exp,
                                             scale=0.2, bias=bexp[:, 3:4])
                        nc.sync.dma_start(out=h2x_mine[ts(w, 128), :], in_=h2row[:])
                        if w == SPLIT_W - 1:
                            nc.gpsimd.collective_compute(
                                "AllGather", OP.bypass, replica_groups=rg,
                                ins=[h2x_mine[0:LOCA, :]],
                                outs=[h2x_full[0:HALF, :]])

            nc.gpsimd.collective_compute(
                "AllGather", OP.bypass, replica_groups=rg,
                ins=[h2x_mine[LOCA:NLOCP, :]], outs=[h2x_full[HALF:NFULL, :]])

            # ---------------- Phase E2: layer-2 edges + pooling ----------------
            with tc.tile_pool(name="e2", bufs=2) as e2, \
                 tc.tile_pool(name="e2s", bufs=2) as e2s, \
                 tc.tile_pool(name="e2t", bufs=3) as e2t, \
                 tc.tile_pool(name="e2ep", bufs=2) as ep2, \
                 tc.tile_pool(name="e2tp", bufs=2, space="PSUM") as e2tp, \
                 tc.tile_pool(name="e2ps", bufs=2, space="PSUM") as e2ps, \
                 tc.tile_pool(name="poolps", bufs=1, space="PSUM") as plps:
                pspool = plps.tile([32, 33], FP32)
                olo = ohi = 0
                ct0 = 0
                wcount = 0
                for s, ws in enumerate(chunks):
                    spans, C = _chunk_cols(T_LO, T_HI, ws)
                    ntl = sum(T_LO[w] for w in ws)
                    nlo, nhi = ntl * 128, (C - ntl) * 128
                    G2 = e2.tile([128, C, 128], BF16, tag="G2")
                    if nlo:
                        ilo = e2.tile([128, nlo // 16], I16, tag="ilo2")
                        nc.sync.dma_start(out=ilo[:], in_=idxlo[:, olo:olo + nlo // 16])
                        nc.gpsimd.dma_gather(
                            out_ap=G2[:, 0:ntl, :], in_ap=h2x_full[0:HALF, :],
                            idxs_ap=ilo[:], num_idxs=nlo, num_idxs_reg=nlo,
                            elem_size=128, single_packet=False, queue_num=0)
                    if nhi:
                        ihi = e2.tile([128, nhi // 16], I16, tag="ihi2")
                        nc.sync.dma_start(out=ihi[:], in_=idxhi[:, ohi:ohi + nhi // 16])
                        nc.gpsimd.dma_gather(
                            out_ap=G2[:, ntl:C, :], in_ap=h2x_full[HALF:NFULL, :],
                            idxs_ap=ihi[:], num_idxs=nhi, num_idxs_reg=nhi,
                            elem_size=128, single_packet=False, queue_num=1)
                    olo += nlo // 16; ohi += nhi // 16

                    S2 = e2s.tile([128, C, 128], BF16, tag="S2")
                    nc.vector.tensor_tensor(
                        out=S2[:],
                        in0=iota_bf[:][:, None, :].to_broadcast([128, C, 128]),
                        in1=drel_sb[:, ct0:ct0 + C][:, :, None].to_broadcast([128, C, 128]),
                        op=OP.is_equal)
                    SttC = e2s.tile([128, C, 128], BF16, tag="SttC2")
                    nc.sync.dma_start(out=SttC[:],
                                      in_=sttoh[:, ct0 * 128:(ct0 + C) * 128])
                    ct0 += C
                    V2 = e2s.tile([128, C, 33], BF16, tag="V2")

                    for w in ws:
                        cols = [t for a0, a1 in spans[w] for t in range(a0, a1)]
                        Dwp2 = e2tp.tile([128, len(cols), 2], FP32, tag="dv")
                        for j, t in enumerate(cols):
                            nc.tensor.matmul(Dwp2[:, j, :], lhsT=SttC[:, t, :],
                                             rhs=dtab2[:, w, :], start=True, stop=True)
                        j0 = 0
                        for a0, a1 in spans[w]:
                            n = a1 - a0
                            if n == 0:
                                continue
                            A2 = e2t.tile([128, n, 2], BF16, tag="A2")
                            nc.vector.tensor_tensor(out=A2[:], in0=G2[:, a0:a1, 32:34],
                                                    in1=Dwp2[:, j0:j0 + n, :], op=OP.mult)
                            nc.vector.tensor_tensor(out=V2[:, a0:a1, 32:33],
                                                    in0=A2[:, :, 0:1], in1=A2[:, :, 1:2],
                                                    op=OP.max)
                            nc.vector.tensor_tensor(
                                out=V2[:, a0:a1, 0:32], in0=G2[:, a0:a1, 0:32],
                                in1=V2[:, a0:a1, 32:33].to_broadcast([128, n, 32]),
                                op=OP.mult)
                            j0 += n
                        psw2 = e2ps.tile([128, 33], FP32, tag="psw2")
                        for j, t in enumerate(cols):
                            nc.tensor.matmul(psw2[:], lhsT=S2[:, t, :], rhs=V2[:, t, :],
                                             start=(j == 0), stop=(j == len(cols) - 1))
                        den2 = ep2.tile([128, 1], FP32, tag="den2")
                        nc.vector.tensor_scalar_max(den2[:], psw2[:, 32:33], 1e-30)
                        rd2 = ep2.tile([128, 1], FP32, tag="rd2")
                        nc.vector.reciprocal(rd2[:], den2[:])
                        z2 = ep2.tile([128, 32], FP32, tag="z2")
                        nc.vector.tensor_scalar(out=z2[:], in0=psw2[:, 0:32],
                                                scalar1=rd2[:, 0:1], scalar2=None,
                                                op0=OP.mult)
                        nc.vector.tensor_tensor(out=z2[:], in0=z2[:], in1=b2sb[:],
                                                op=OP.add)
                        r2 = ep2.tile([128, 32], FP32, tag="r2")
                        nc.scalar.activation(r2[:], z2[:], AF.Relu)
                        t2 = ep2.tile([128, 32], FP32, tag="t2")
                        nc.scalar.activation(t2[:], z2[:], AF.Relu, scale=-1.0)
                        e2_ = ep2.tile([128, 32], FP32, tag="e2_")
                        nc.scalar.activation(e2_[:], t2[:], AF.Exp, scale=-1.0)
                        h3a = ep2.tile([128, 33], FP32, tag="h3a")
                        nc.vector.tensor_tensor(out=h3a[:, 0:32], in0=r2[:],
                                                in1=e2_[:], op=OP.add)
                        nc.vector.memset(h3a[:, 32:33], 1.0)
                        sbt = ep2.tile([128, 32], FP32, tag="sbt")
                        nc.sync.dma_start(out=sbt[:], in_=Sb[ts(w, 128), :])
                        nc.tensor.matmul(pspool[:], lhsT=sbt[:], rhs=h3a[:],
                                         start=(wcount == 0), stop=(wcount == NW - 1))
                        wcount += 1

                # ---------------- Phase F: AllReduce + MLP ----------------
                poolsb = ep2.tile([32, 33], FP32)
                nc.vector.tensor_copy(out=poolsb[:], in_=pspool[:])
                nc.sync.dma_start(out=pool_mine[:], in_=poolsb[:])
                nc.gpsimd.collective_compute(
                    "AllReduce", OP.add, replica_groups=rg,
                    ins=[pool_mine[:]], outs=[pool_sum[:]])
                psf = ep2.tile([32, 33], FP32)
                nc.sync.dma_start(out=psf[:], in_=pool_sum[:])
                cntc = ep2.tile([32, 1], FP32)
                nc.vector.tensor_scalar_max(cntc[:], psf[:, 32:33], 1.0)
                rc = ep2.tile([32, 1], FP32)
                nc.vector.reciprocal(rc[:], cntc[:])
                gv = ep2.tile([32, 32], FP32)
                nc.vector.tensor_scalar(out=gv[:], in0=psf[:, 0:32],
                                        scalar1=rc[:, 0:1], scalar2=-1.0,
                                        op0=OP.mult, op1=OP.add)
                ptg = e2ps.tile([32, 32], FP32, tag="mlp", bufs=1)
                nc.tensor.transpose(ptg[:], gv[:], idf[0:32, 0:32])
                gvT = ep2.tile([32, 32], FP32)
                nc.vector.tensor_copy(out=gvT[:], in_=ptg[:])
                psh = e2ps.tile([16, 32], FP32, tag="mlp", bufs=1)
                nc.tensor.matmul(psh[:], lhsT=wc1sb[:], rhs=gvT[:],
                                 start=True, stop=True)
                hidT = ep2.tile([16, 32], FP32)
                nc.scalar.activation(hidT[:], psh[:], AF.Relu, bias=bc1sb[:])
                psr = e2ps.tile([1, 32], FP32, tag="mlp", bufs=1)
                nc.tensor.matmul(psr[:], lhsT=wc2sb[:], rhs=hidT[:],
                                 start=True, stop=True)
                rsb = ep2.tile([1, 32], FP32)
                nc.scalar.activation(rsb[:], psr[:], AF.Copy, bias=bc2f)
                nc.sync.dma_start(out=riskT[:], in_=rsb[:])

    nc.compile()
    return nc


def kernel(**inputs):
    global last_results
    x = np.asarray(inputs["x"], np.float32)
    ei = np.asarray(inputs["edge_index"])
    batch = np.asarray(inputs["batch"]).astype(np.int64)
    W1 = np.asarray(inputs["W1"], np.float32)
    as1 = np.asarray(inputs["att_src1"], np.float32)
    ad1 = np.asarray(inputs["att_dst1"], np.float32)
    b1 = np.asarray(inputs["b1"], np.float32)
    W2 = np.asarray(inputs["W2"], np.float32)
    as2 = np.asarray(inputs["att_src2"], np.float32)
    ad2 = np.asarray(inputs["att_dst2"], np.float32)
    b2 = np.asarray(inputs["b2"], np.float32)
    Wc1 = np.asarray(inputs["Wc1"], np.float32)
    bc1 = np.asarray(inputs["bc1"], np.float32)
    Wc2 = np.asarray(inputs["Wc2"], np.float32)
    bc2 = np.asarray(inputs["bc2"], np.float32)

    T_LO, T_HI, chunks, percore, node_core, node_local = _prep_edges(ei)
    CT = int((T_LO + T_HI).sum())
    LOCOLS = int(T_LO.sum()) * 8
    HICOLS = int(T_HI.sum()) * 8

    A_s = np.zeros((256, 8), np.float32)
    A_d = np.zeros((256, 8), np.float32)
    for h in range(H):
        A_s[h * 32:(h + 1) * 32, h] = as1[h]
        A_d[h * 32:(h + 1) * 32, h] = ad1[h]
    W1ext = np.hstack([W1, W1 @ A_s, W1 @ A_d]).astype(bf16)
    W2ext = np.hstack([W2, W2 @ as2[0][:, None], W2 @ ad2[0][:, None]]).astype(bf16)
    c2 = np.ones(256, np.float32) @ W2ext.astype(np.float32)  # [34]

    nc = _build_program(T_LO, T_HI, chunks, CT, LOCOLS, HICOLS,
                        float(c2[32]), float(c2[33]), float(bc2.ravel()[0]))

    in_maps = []
    for c in range(NCORES):
        lo, hi, dr = percore[c]
        mine = node_core == c
        locs = node_local[mine]
        xs = np.zeros((256, NLOCP), bf16)
        xs[:, locs] = x[mine].T.astype(bf16)
        Sbm = np.zeros((NLOCP, 32), np.float32)
        Sbm[locs, batch[mine]] = 1.0
        dr2 = dr.reshape(CT, 128)
        oh = (dr2[:, None, :] == np.arange(128)[None, :, None])
        sttv = np.ascontiguousarray(
            oh.transpose(1, 0, 2).reshape(128, CT * 128)).astype(bf16)
        in_maps.append({
            "xT": np.ascontiguousarray(xs),
            "idxlo": _wrap_idx(lo), "idxhi": _wrap_idx(hi),
            "dstrel": np.ascontiguousarray(dr.reshape(CT, 128).T.astype(bf16)),
            "sttoh": sttv,
            "Sb": Sbm,
            "W1e": W1ext, "W2e": W2ext,
            "b1r": np.ascontiguousarray(np.broadcast_to(b1, (128, 256))).astype(np.float32),
            "b2r": np.ascontiguousarray(np.broadcast_to(b2, (128, 32))).astype(np.float32),
            "c2r": np.ascontiguousarray(np.broadcast_to(c2[0:32], (128, 32))).astype(np.float32),
            "bc1t": bc1.reshape(16, 1).astype(np.float32),
            "Wc1t": Wc1, "Wc2t": Wc2.reshape(16, 1),
        })
    res = run_bass_kernel_spmd(nc, in_maps, core_ids=list(range(NCORES)))
    last_results = res
    return res.results[0]["riskT"].reshape(32, 1).astype(np.float32)
